# revision 21
# baseline (speedup 1.0000x reference)
"""Trainium2 Bass kernel for 16-head MHA with RoPE (dense_transformer).

Sharding: tensor-parallel over heads (2 heads/core on 8 cores) for
QKV projection + attention, then an AllToAll resharding to
token-parallel (512 tokens/core) for the output projection.

v6 layout strategy (per core, rank r):
  - The full dim-major activation xT [1024, 4096] is fed to every core
    directly as a kernel input (host transposes once, bf16). This
    removes the AllGather + on-device transpose stage of v1 (~310us).
  - q/k are computed into one interleaved tile qkt [128, 2, L] f32 via
    bf16  wT.T @ xT  matmuls (fp32 PSUM accumulate), per-batch tiles.
    The first two projection chunks are half-sized so PE starts early.
  - RoPE: DVE multiplies by cos and a sign-folded sin table; the
    rotate_half partition swap is 4 plain strip DMAs (issued on the
    idle SP/ACT sequencers, q+k merged) into a scratch tile, then one
    2x-mode DVE add. No slow Pool-issued accumulate-DMAs.
  - v is evacuated bf16 and re-transposed to token-major [keys, 64|1]
    tiles by one xbar transpose-DMA per (batch, head); the appended
    ones column makes  out.T = [v | 1].T @ exp(S.T)  yield the softmax
    denominator as row 64 for free.
  - Attention is ScalarE(exp)-bound: batch-0/head-0 attention runs in
    small FQ=512 PSUM pools (4 banks) CONCURRENTLY with batch-1
    projection (4 banks); the remaining blocks use FQ=1024 (8 banks).
  - Softmax normalization: DVE reciprocal of the denominator row,
    GpSimd partition_broadcast (idle engine), DVE multiply -> bf16.
  - The head-parallel -> token-parallel reshard is TWO AllToAlls in
    bf16: the head-0 A2A overlaps head-1's attention; the o-projection
    is split so its head-0 half runs under the head-1 A2A (wo rows are
    host-permuted into a2a-output order to keep K-chunks contiguous).
"""

import numpy as np

# Problem shape (hardcoded per contract - kernel.py must be self-contained)
B, L_FULL, D = 2, 2048, 1024
H, HD = 16, 64
N_CORES = 8
HPC = H // N_CORES            # heads per core = 2
KC = D // 128                 # contraction chunks = 8


def _rope_tables(L):
    inv_freq = 1.0 / (10000.0 ** (np.arange(0, HD, 2, dtype=np.float64) / HD))
    t = np.arange(L, dtype=np.float64)
    freqs = np.outer(t, inv_freq)                      # [L, 32]
    emb = np.concatenate([freqs, freqs], -1)           # [L, 64]
    cos_t = np.cos(emb).T.astype(np.float32)           # [64, L]
    sin_t = np.sin(emb).T.astype(np.float32)
    cost = np.concatenate([cos_t, cos_t], 0)           # [128, L] (2 heads)
    sp = np.concatenate([sin_t[:32], -sin_t[32:]], 0)  # sign-folded
    sinp = np.concatenate([sp, sp], 0)                 # [128, L]
    return np.ascontiguousarray(cost), np.ascontiguousarray(sinp)


def build_mha(tc, L=L_FULL, debug=False):
    """Emit the MHA program into TileContext `tc`.

    Declares its own DRAM I/O tensors:
      in : xt [D, B*L] bf16 (full, replicated), wqt/wkt/wvt [D, 128] bf16,
           wot [D, D] bf16 (rows permuted to a2a-output order)
      out: y [B*L/8, D] f32
    """
    import concourse.bass as bass
    import concourse.mybir as mybir
    from contextlib import ExitStack

    nc = tc.nc
    f32 = mybir.dt.float32
    f32r = mybir.dt.float32r
    bf16 = mybir.dt.bfloat16
    AF = mybir.ActivationFunctionType
    ALU = mybir.AluOpType

    T = B * L                     # tokens
    TPC = T // N_CORES            # tokens per core (a2a shard width)
    MC = L // 128                 # key chunks per batch
    FQE = min(512, L)             # early-attention query tile (4 PSUM banks)
    FQM = min(1024, L)            # main attention query tile (8 PSUM banks)
    KH = N_CORES * 64 // 128      # o-proj K-chunks per head half = 4
    MT = min(128, TPC)            # o-proj token tile
    scale = float(HD) ** -0.5
    rg = [list(range(N_CORES))]

    # projection chunk schedule (within-batch offsets): the first chunks of
    # batch 0 are half-sized so the first matmul fires as early as possible.
    if L >= 512:
        sched0 = [(0, 256), (256, 256)] + [(o, 512) for o in range(512, L, 512)]
        schedN = [(o, 512) for o in range(0, L, 512)]
    else:
        sched0 = [(0, L)]
        schedN = [(0, L)]
    chunks = [(b, o, ch) for b in range(B)
              for (o, ch) in (sched0 if b == 0 else schedN)]

    def r(ap):
        return ap.bitcast(f32r)

    # ---- I/O ----
    xt_d = nc.dram_tensor("xt", [D, T], bf16, kind="ExternalInput").ap()
    wqt_d = nc.dram_tensor("wqt", [D, 128], bf16, kind="ExternalInput").ap()
    wkt_d = nc.dram_tensor("wkt", [D, 128], bf16, kind="ExternalInput").ap()
    wvt_d = nc.dram_tensor("wvt", [D, 128], bf16, kind="ExternalInput").ap()
    wot_d = nc.dram_tensor("wot", [D, D], bf16, kind="ExternalInput").ap()
    y_d = nc.dram_tensor("y", [TPC, D], f32, kind="ExternalOutput").ap()
    if debug:
        dbg_qkt = nc.dram_tensor("dbg_qkt", [128, 2, L], f32, kind="ExternalOutput").ap()
        dbg_vsb = nc.dram_tensor("dbg_vsb", [128, HPC, MC, 65], f32, kind="ExternalOutput").ap()
        dbg_ao0 = nc.dram_tensor("dbg_ao0", [N_CORES * 64, TPC], f32, kind="ExternalOutput").ap()
        dbg_ao1 = nc.dram_tensor("dbg_ao1", [N_CORES * 64, TPC], f32, kind="ExternalOutput").ap()
        dbg_at = nc.dram_tensor("dbg_at", [128, KC, TPC], f32, kind="ExternalOutput").ap()

    # ---- inline constants ----
    cost_np, sinp_np = _rope_tables(L)
    cost_d = nc.inline_tensor(cost_np, name="cost")
    sinp_d = nc.inline_tensor(sinp_np, name="sinp")

    ctx = ExitStack()
    with ctx:
        # ---------------- persistent pools ----------------
        wpool = ctx.enter_context(tc.tile_pool(name="wqkv", bufs=1))
        wq_sb = wpool.tile([128, KC, 128], bf16)
        wk_sb = wpool.tile([128, KC, 128], bf16)
        wv_sb = wpool.tile([128, KC, 128], bf16)
        nc.sync.dma_start(wq_sb[:], wqt_d.rearrange("(kk p) c -> p kk c", kk=KC))
        nc.scalar.dma_start(wk_sb[:], wkt_d.rearrange("(kk p) c -> p kk c", kk=KC))
        nc.gpsimd.dma_start(wv_sb[:], wvt_d.rearrange("(kk p) c -> p kk c", kk=KC))

        cpool = ctx.enter_context(tc.tile_pool(name="consts", bufs=1))
        cost = cpool.tile([128, L], f32)
        sinp = cpool.tile([128, L], f32)

        dram = ctx.enter_context(tc.tile_pool(name="dram", bufs=1, space="DRAM"))
        a2a_in = [dram.tile([N_CORES * 64, TPC], bf16, name=f"a2ai{h}")
                  for h in range(HPC)]
        a2a_out = [dram.tile([N_CORES * 64, TPC], bf16, name=f"a2ao{h}")
                   for h in range(HPC)]

        qkpool = ctx.enter_context(tc.tile_pool(name="qk", bufs=1))
        # per-batch tiles so batch-0 attention can overlap batch-1 work;
        # q and k interleaved so one strip DMA serves both
        qkt = [qkpool.tile([128, 2, L], f32r, name=f"qkt{b}") for b in range(B)]
        vt = [qkpool.tile([64, HPC, L], bf16, name=f"vt{b}") for b in range(B)]

        vpool = ctx.enter_context(tc.tile_pool(name="vtm", bufs=1))
        v_sb = [vpool.tile([128, HPC, MC, 65], bf16, tag=f"v{b}", name=f"v_sb{b}")
                for b in range(B)]
        vdp = ctx.enter_context(tc.tile_pool(name="vdense", bufs=2))

        # o-proj tiles (at_sb half-loads are emitted mid-stream)
        wop = ctx.enter_context(tc.tile_pool(name="wo", bufs=1))
        wo_sb = wop.tile([128, KC, D], bf16)
        atp = ctx.enter_context(tc.tile_pool(name="at", bufs=1))
        at_sb = [atp.tile([128, KH, TPC], bf16, name=f"at_sb{h}")
                 for h in range(HPC)]

        # attention epilogue + pt pools (shared by early + main attention)
        epool = ctx.enter_context(tc.tile_pool(name="ep", bufs=2))
        ptpool = ctx.enter_context(tc.tile_pool(name="pt", bufs=4))

        def attn_block(h, b, nh, FQ, stp, oup):
            """One (head, batch, qtile) attention block: scores, exp, A@V,
            normalize, stage into a2a_in[h]."""
            hs = slice(h * 64, (h + 1) * 64)
            NQ = min(512, FQ)
            q0 = nh * FQ
            outU = oup.tile([65, FQ], f32, tag="outU")
            for m in range(MC):
                ks = slice(m * 128, (m + 1) * 128)
                st = stp.tile([128, FQ], f32, tag="st")
                for n in range(FQ // NQ):
                    qs = slice(q0 + n * NQ, q0 + (n + 1) * NQ)
                    nc.tensor.matmul(st[:, n * NQ:(n + 1) * NQ],
                                     qkt[b][hs, 1, ks], qkt[b][hs, 0, qs],
                                     start=True, stop=True)
                pt = ptpool.tile([128, FQ], bf16, tag="pt")
                nc.scalar.activation(pt[:], st[:], AF.Exp, scale=scale)
                for n in range(FQ // NQ):
                    ns = slice(n * NQ, (n + 1) * NQ)
                    nc.tensor.matmul(outU[:, ns],
                                     v_sb[b][:, h, m, :], pt[:, ns],
                                     start=(m == 0), stop=(m == MC - 1))
            # epilogue: normalize (DVE+Pool, off the PE/ACT path)
            dsb = epool.tile([1, FQ], f32, tag="dsb")
            nc.vector.reciprocal(dsb[:], outU[64:65, :])
            bc_sb = epool.tile([64, FQ], f32, tag="bc_sb")
            nc.gpsimd.partition_broadcast(bc_sb[:], dsb[:])
            a_st = epool.tile([64, FQ], bf16, tag="a_st")
            nc.vector.tensor_mul(a_st[:], outU[0:64, :], bc_sb[:])
            for nq in range(FQ // TPC):
                j = (b * L + nh * FQ) // TPC + nq
                nc.sync.dma_start(
                    a2a_in[h][j * 64:(j + 1) * 64, :],
                    a_st[:, nq * TPC:(nq + 1) * TPC])

        # ---------------- projections (+RoPE) ----------------
        pstack = ExitStack()
        xtp = pstack.enter_context(tc.tile_pool(name="xt", bufs=2))
        up = pstack.enter_context(tc.tile_pool(name="u", bufs=2))
        t1p = pstack.enter_context(tc.tile_pool(name="t1", bufs=2))
        rup = pstack.enter_context(tc.tile_pool(name="ru", bufs=2))
        pps = pstack.enter_context(tc.tile_pool(name="pps", bufs=1, space="PSUM"))
        onc = pstack.enter_context(tc.tile_pool(name="onc", bufs=1))
        ones_col = onc.tile([128, HPC, MC, 1], bf16)
        nc.gpsimd.memset(ones_col[:], 1.0)

        xtiles = {}

        def load_chunk(i):
            b, l0, ch = chunks[i]
            g0 = b * L + l0
            xt_c = xtp.tile([128, KC, 512], bf16, tag="xt_c", name=f"xt_c{i}")
            xsrc = xt_d[:, g0:g0 + ch].rearrange("(kk p) c -> p kk c", kk=KC)
            nc.sync.dma_start(xt_c[:, :, 0:ch], xsrc)
            xtiles[i] = xt_c

        def proj_chunk(i):
            b, l0, ch = chunks[i]
            sl = slice(l0, l0 + ch)
            xt_c = xtiles.pop(i)
            q_ps = pps.tile([128, 512], f32, tag="q_ps")
            k_ps = pps.tile([128, 512], f32, tag="k_ps")
            v_ps = pps.tile([128, 512], f32, tag="v_ps")
            for kk in range(KC):
                st_, sp_ = (kk == 0), (kk == KC - 1)
                nc.tensor.matmul(q_ps[:, 0:ch], wq_sb[:, kk, :], xt_c[:, kk, 0:ch],
                                 start=st_, stop=sp_)
                nc.tensor.matmul(k_ps[:, 0:ch], wk_sb[:, kk, :], xt_c[:, kk, 0:ch],
                                 start=st_, stop=sp_)
                nc.tensor.matmul(v_ps[:, 0:ch], wv_sb[:, kk, :], xt_c[:, kk, 0:ch],
                                 start=st_, stop=sp_)
            # RoPE evacuation: cos/sin muls on DVE, rotate_half via 4 plain
            # strip DMAs (SP/ACT issue) + one 2x-mode DVE add.
            tb = slice(l0, l0 + ch)
            u = up.tile([128, 2, 512], f32, tag="u")
            t1 = t1p.tile([128, 2, 512], f32, tag="t1")
            ru = rup.tile([128, 2, 512], f32, tag="ru")
            nc.vector.tensor_mul(u[:, 0, 0:ch], q_ps[:, 0:ch], sinp[:, tb])
            nc.vector.tensor_mul(u[:, 1, 0:ch], k_ps[:, 0:ch], sinp[:, tb])
            nc.vector.tensor_mul(t1[:, 0, 0:ch], q_ps[:, 0:ch], cost[:, tb])
            nc.vector.tensor_mul(t1[:, 1, 0:ch], k_ps[:, 0:ch], cost[:, tb])
            for hh in range(HPC):
                a, b_ = hh * 64, hh * 64 + 32
                c_ = hh * 64 + 64
                eng = nc.sync if hh == 0 else nc.gpsimd
                eng.dma_start(ru[a:b_, :, 0:ch], u[b_:c_, :, 0:ch])
                eng.dma_start(ru[b_:c_, :, 0:ch], u[a:b_, :, 0:ch])
            with nc.allow_low_precision(reason="fp32r rounding on RoPE add"):
                nc.vector.tensor_add(qkt[b][:, :, sl], t1[:, :, 0:ch],
                                     ru[:, :, 0:ch])
            for hh in range(HPC):
                nc.vector.tensor_copy(vt[b][:, hh, sl],
                                      v_ps[hh * 64:(hh + 1) * 64, 0:ch])

            # v -> token-major [keys, 64|1] once this batch is projected:
            # one xbar transpose-DMA per head (token t lands at partition
            # t%128, chunk t//128 -- exactly the m-chunk layout).
            if l0 + ch == L:
                nc.vector.tensor_copy(v_sb[b][:, :, :, 64:65],
                                      ones_col[:, :, :, :])
                for hh in range(HPC):
                    # HW xbar transpose requires a dense output; copy into
                    # the 65-stride layout with one 4x-mode DVE op after.
                    vden = vdp.tile([128, MC, 64], bf16, tag="vden",
                                    name=f"vden{b}_{hh}")
                    nc.sync.dma_start_transpose(vden[:], vt[b][:, hh, :])
                    nc.vector.tensor_copy(v_sb[b][:, hh, :, 0:64], vden[:])

        NC0 = len(sched0)             # chunks in batch 0
        load_chunk(0)
        if len(chunks) > 1:
            load_chunk(1)
        # rope tables arrive after the first x chunks
        nc.sync.dma_start(cost[:], cost_d.ap()[:, :])
        nc.scalar.dma_start(sinp[:], sinp_d.ap()[:, :])

        for i in range(NC0):
            if i + 2 < len(chunks):
                load_chunk(i + 2)
            proj_chunk(i)

        # early attention: (h0, b0) in 4-bank pools, concurrent with the
        # batch-1 projection (which uses the other 4 PSUM banks).
        estack = ExitStack()
        stpE = estack.enter_context(tc.tile_pool(name="stpE", bufs=3, space="PSUM"))
        oupE = estack.enter_context(tc.tile_pool(name="oupE", bufs=2, space="PSUM"))
        for eh in range(HPC):
            for nh in range(L // FQE):
                attn_block(eh, 0, nh, FQE, stpE, oupE)

        for i in range(NC0, len(chunks)):
            if i + 2 < len(chunks):
                load_chunk(i + 2)
            proj_chunk(i)

        estack.close()
        pstack.close()

        # o-proj weights: needed last; SP is idle during attention
        nc.sync.dma_start(wo_sb[:],
                          wot_d.rearrange("(kk p) c -> p kk c", kk=KC))

        # ---------------- main attention (head-outer) ----------------
        s5 = ExitStack()
        stp = s5.enter_context(tc.tile_pool(name="stp", bufs=2, space="PSUM"))
        oup = s5.enter_context(tc.tile_pool(name="oup", bufs=2, space="PSUM"))

        blocks = [(hh, bb) for hh in range(HPC) for bb in range(1, B)]
        done_h = {hh: L // FQE for hh in range(HPC)}   # batch 0 done early
        total_h = {hh: L // FQE + (B - 1) * (L // FQM) for hh in range(HPC)}
        for h, b in blocks:
            for nh in range(L // FQM):
                attn_block(h, b, nh, FQM, stp, oup)
            done_h[h] = done_h.get(h, 0) + L // FQM
            if done_h[h] == total_h[h]:
                # all batches of this head staged -> reshard while the rest
                # of attention runs
                nc.gpsimd.collective_compute(
                    "AllToAll", ALU.bypass,
                    ins=[a2a_in[h].opt()], outs=[a2a_out[h].opt()],
                    replica_groups=rg,
                )
                nc.sync.dma_start(
                    at_sb[h][:],
                    a2a_out[h][:, :].rearrange("(j p) t -> p j t", j=KH))

        s5.close()

        if debug:
            with tc.tile_pool(name="dbgp", bufs=1) as dbgp:
                dcp = dbgp.tile([128, 2, L], f32)
                nc.vector.tensor_copy(dcp[:], qkt[0][:, :, :].bitcast(f32))
                nc.sync.dma_start(dbg_qkt[:, :, :], dcp[:])
                dv = dbgp.tile([128, HPC, MC, 65], f32)
                nc.vector.tensor_copy(dv[:], v_sb[0][:, :, :, :])
                nc.sync.dma_start(dbg_vsb[:, :, :, :], dv[:])
                da = dbgp.tile([128, KC, TPC], f32)
                nc.vector.tensor_copy(da[:, 0:KH, :], at_sb[0][:, :, :])
                nc.vector.tensor_copy(da[:, KH:KC, :], at_sb[1][:, :, :])
                nc.sync.dma_start(dbg_at[:, :, :], da[:])
                for hh in range(HPC):
                    dao = dbgp.tile([64, N_CORES, TPC], f32, tag="dao", name=f"dao{hh}")
                    nc.gpsimd.dma_start(
                        dao[:],
                        a2a_out[hh][:, :].rearrange("(c p) t -> p c t", c=N_CORES))
                    nc.sync.dma_start(
                        [dbg_ao0, dbg_ao1][hh].rearrange("(c p) t -> p c t", c=N_CORES),
                        dao[:])

        # ---------------- o-projection (bf16, wo rows permuted) ----------
        # Split into a head-0 pass (runs under the head-1 AllToAll) and a
        # head-1 pass; y_ps tiles for all output blocks stay resident.
        with tc.tile_pool(name="ysb", bufs=2) as ysp, \
             tc.tile_pool(name="yps", bufs=1, space="PSUM") as yps:
            nyo = D // 512
            y_ps = {}
            for mt in range(TPC // MT):
                for no in range(nyo):
                    y_ps[(mt, no)] = yps.tile([MT, 512], f32, tag=f"y{mt}_{no}",
                                              name=f"y_ps{mt}_{no}")
            for h in range(HPC):
                for mt in range(TPC // MT):
                    ms = slice(mt * MT, (mt + 1) * MT)
                    for j in range(KH):
                        kk = h * KH + j
                        for no in range(nyo):
                            nc.tensor.matmul(
                                y_ps[(mt, no)][:], at_sb[h][:, j, ms],
                                wo_sb[:, kk, no * 512:(no + 1) * 512],
                                start=(kk == 0), stop=(kk == KC - 1))
            for mt in range(TPC // MT):
                ms = slice(mt * MT, (mt + 1) * MT)
                y_sb = ysp.tile([MT, D], f32, tag="y_sb")
                for no in range(nyo):
                    nc.vector.tensor_copy(y_sb[:, no * 512:(no + 1) * 512],
                                          y_ps[(mt, no)][:])
                nc.sync.dma_start(y_d[ms, :], y_sb[:])

    return nc


def make_in_maps(x, wq, wk, wv, wo, L=L_FULL):
    import ml_dtypes
    bf = ml_dtypes.bfloat16
    T = B * L
    x2 = np.asarray(x, dtype=np.float32).reshape(T, D)
    xt_full = np.ascontiguousarray(x2.T.astype(bf))    # [D, T] bf16
    wq = np.asarray(wq, dtype=np.float32)
    wk = np.asarray(wk, dtype=np.float32)
    wv = np.asarray(wv, dtype=np.float32)
    wo = np.asarray(wo, dtype=np.float32)
    # Permute wo^T rows into a2a-output order: head-0 halves of all cores
    # first (64 rows per core), then head-1 halves.
    perm = np.empty(D, dtype=np.int64)
    for h in range(HPC):
        for rk in range(N_CORES):
            i0 = h * (N_CORES * 64) + rk * 64
            perm[i0:i0 + 64] = np.arange(rk * 128 + h * 64,
                                         rk * 128 + h * 64 + 64)
    wot_bf = np.ascontiguousarray(wo.T[perm]).astype(bf)
    in_maps = []
    for rk in range(N_CORES):
        rows = slice(rk * 128, (rk + 1) * 128)
        in_maps.append({
            "xt": xt_full,
            "wqt": np.ascontiguousarray(wq[rows].T).astype(bf),
            "wkt": np.ascontiguousarray(wk[rows].T).astype(bf),
            "wvt": np.ascontiguousarray(wv[rows].T).astype(bf),
            "wot": wot_bf,
        })
    return in_maps


_BUILT = {}


def _get_nc(L=L_FULL):
    if L not in _BUILT:
        import concourse.tile as tile
        from concourse import bacc
        nc = bacc.Bacc(num_devices=N_CORES)
        with tile.TileContext(nc) as tc:
            build_mha(tc, L=L)
        nc.compile()
        _BUILT[L] = nc
    return _BUILT[L]


def kernel(x, wq, wk, wv, wo):
    from concourse.bass_utils import run_bass_kernel_spmd
    nc = _get_nc()
    in_maps = make_in_maps(x, wq, wk, wv, wo)
    res = run_bass_kernel_spmd(nc, in_maps, core_ids=list(range(N_CORES)))
    y = np.concatenate([res.results[rk]["y"] for rk in range(N_CORES)], axis=0)
    return y.reshape(B, L_FULL, D)


# revision 34
# speedup vs baseline: 1.9976x; 1.9976x over previous
"""Trainium2 Bass kernel for 16-head MHA with RoPE (dense_transformer).

Sharding: tensor-parallel over heads (2 heads/core on 8 cores) for
QKV projection + attention, then an AllToAll resharding to
token-parallel (512 tokens/core) for the output projection.

v6 layout strategy (per core, rank r):
  - The full dim-major activation xT [1024, 4096] is fed to every core
    directly as a kernel input (host transposes once, bf16). This
    removes the AllGather + on-device transpose stage of v1 (~310us).
  - q/k are computed into one interleaved tile qkt [128, 2, L] f32 via
    bf16  wT.T @ xT  matmuls (fp32 PSUM accumulate), per-batch tiles.
    The first two projection chunks are half-sized so PE starts early.
  - RoPE: DVE multiplies by cos and a sign-folded sin table; the
    rotate_half partition swap is 4 plain strip DMAs (issued on the
    idle SP/ACT sequencers, q+k merged) into a scratch tile, then one
    2x-mode DVE add. No slow Pool-issued accumulate-DMAs.
  - v is evacuated bf16 and re-transposed to token-major [keys, 64|1]
    tiles by one xbar transpose-DMA per (batch, head); the appended
    ones column makes  out.T = [v | 1].T @ exp(S.T)  yield the softmax
    denominator as row 64 for free.
  - Attention is ScalarE(exp)-bound: batch-0/head-0 attention runs in
    small FQ=512 PSUM pools (4 banks) CONCURRENTLY with batch-1
    projection (4 banks); the remaining blocks use FQ=1024 (8 banks).
  - Softmax normalization: DVE reciprocal of the denominator row,
    GpSimd partition_broadcast (idle engine), DVE multiply -> bf16.
  - The head-parallel -> token-parallel reshard is TWO AllToAlls in
    bf16: the head-0 A2A overlaps head-1's attention; the o-projection
    is split so its head-0 half runs under the head-1 A2A (wo rows are
    host-permuted into a2a-output order to keep K-chunks contiguous).
"""

import numpy as np

# Problem shape (hardcoded per contract - kernel.py must be self-contained)
B, L_FULL, D = 2, 2048, 1024
H, HD = 16, 64
N_CORES = 8
HPC = H // N_CORES            # heads per core = 2
KC = D // 128                 # contraction chunks = 8


def _rope_tables(L):
    inv_freq = 1.0 / (10000.0 ** (np.arange(0, HD, 2, dtype=np.float64) / HD))
    t = np.arange(L, dtype=np.float64)
    freqs = np.outer(t, inv_freq)                      # [L, 32]
    emb = np.concatenate([freqs, freqs], -1)           # [L, 64]
    cos_t = np.cos(emb).T.astype(np.float32)           # [64, L]
    sin_t = np.sin(emb).T.astype(np.float32)
    cost = np.concatenate([cos_t, cos_t], 0)           # [128, L] (2 heads)
    sp = np.concatenate([sin_t[:32], -sin_t[32:]], 0)  # sign-folded
    sinp = np.concatenate([sp, sp], 0)                 # [128, L]
    return np.ascontiguousarray(cost), np.ascontiguousarray(sinp)


def build_mha(tc, L=L_FULL, debug=False):
    """Emit the MHA program into TileContext `tc`.

    Declares its own DRAM I/O tensors:
      in : xt [D, B*L] bf16 (full, replicated), wqt/wkt/wvt [D, 128] bf16,
           wot [D, D] bf16 (rows permuted to a2a-output order)
      out: y [B*L/8, D] f32
    """
    import concourse.bass as bass
    import concourse.mybir as mybir
    from contextlib import ExitStack

    nc = tc.nc
    f32 = mybir.dt.float32
    f32r = mybir.dt.float32r
    bf16 = mybir.dt.bfloat16
    AF = mybir.ActivationFunctionType
    ALU = mybir.AluOpType

    T = B * L                     # tokens
    TPC = T // N_CORES            # tokens per core (a2a shard width)
    MC = L // 128                 # key chunks per batch
    FQE = min(512, L)             # early-attention query tile (4 PSUM banks)
    FQM = min(1024, L)            # main attention query tile (8 PSUM banks)
    KH = N_CORES * 64 // 128      # o-proj K-chunks per head half = 4
    MT = min(128, TPC)            # o-proj token tile
    scale = float(HD) ** -0.5
    rg = [list(range(N_CORES))]

    # projection chunk schedule (within-batch offsets): the first chunks of
    # batch 0 are half-sized so the first matmul fires as early as possible.
    if L >= 512:
        sched0 = [(0, 256), (256, 256)] + [(o, 512) for o in range(512, L, 512)]
        schedN = [(o, 512) for o in range(0, L, 512)]
    else:
        sched0 = [(0, L)]
        schedN = [(0, L)]
    chunks = [(b, o, ch) for b in range(B)
              for (o, ch) in (sched0 if b == 0 else schedN)]

    def r(ap):
        return ap.bitcast(f32r)

    # ---- I/O ----
    xt_d = nc.dram_tensor("xt", [D, T], bf16, kind="ExternalInput").ap()
    wqt_d = nc.dram_tensor("wqt", [D, 128], bf16, kind="ExternalInput").ap()
    wkt_d = nc.dram_tensor("wkt", [D, 128], bf16, kind="ExternalInput").ap()
    wvt_d = nc.dram_tensor("wvt", [D, 128], bf16, kind="ExternalInput").ap()
    wot_d = nc.dram_tensor("wot", [D, D], bf16, kind="ExternalInput").ap()
    y_d = nc.dram_tensor("y", [TPC, D], f32, kind="ExternalOutput").ap()
    if debug:
        dbg_qkt = nc.dram_tensor("dbg_qkt", [128, 2, L], f32, kind="ExternalOutput").ap()
        dbg_vsb = nc.dram_tensor("dbg_vsb", [128, HPC, MC, 65], f32, kind="ExternalOutput").ap()
        dbg_ao0 = nc.dram_tensor("dbg_ao0", [N_CORES * 64, TPC], f32, kind="ExternalOutput").ap()
        dbg_ao1 = nc.dram_tensor("dbg_ao1", [N_CORES * 64, TPC], f32, kind="ExternalOutput").ap()
        dbg_at = nc.dram_tensor("dbg_at", [128, KC, TPC], f32, kind="ExternalOutput").ap()

    # ---- inline constants ----
    import ml_dtypes
    cost_np, sinp_np = _rope_tables(L)
    cost_d = nc.inline_tensor(cost_np.astype(ml_dtypes.bfloat16), name="cost")
    sinp_d = nc.inline_tensor(sinp_np.astype(ml_dtypes.bfloat16), name="sinp")

    ctx = ExitStack()
    with ctx:
        # ---------------- persistent pools ----------------
        wpool = ctx.enter_context(tc.tile_pool(name="wqkv", bufs=1))
        wq_sb = wpool.tile([128, KC, 128], bf16)
        wk_sb = wpool.tile([128, KC, 128], bf16)
        wv_sb = wpool.tile([128, KC, 128], bf16)
        nc.sync.dma_start(wq_sb[:], wqt_d.rearrange("(kk p) c -> p kk c", kk=KC))
        nc.scalar.dma_start(wk_sb[:], wkt_d.rearrange("(kk p) c -> p kk c", kk=KC))
        nc.gpsimd.dma_start(wv_sb[:], wvt_d.rearrange("(kk p) c -> p kk c", kk=KC))

        cpool = ctx.enter_context(tc.tile_pool(name="consts", bufs=1))
        cost = cpool.tile([128, L], bf16)
        sinp = cpool.tile([128, L], bf16)

        dram = ctx.enter_context(tc.tile_pool(name="dram", bufs=1, space="DRAM"))
        a2a_in = [dram.tile([N_CORES * 64, TPC], bf16, name=f"a2ai{h}")
                  for h in range(HPC)]
        a2a_out = [dram.tile([N_CORES * 64, TPC], bf16, name=f"a2ao{h}")
                   for h in range(HPC)]

        qkpool = ctx.enter_context(tc.tile_pool(name="qk", bufs=1))
        # per-batch tiles so batch-0 attention can overlap batch-1 work;
        # q and k interleaved so one strip DMA serves both
        qkt = [qkpool.tile([128, 2, L], f32r, name=f"qkt{b}") for b in range(B)]
        vt = [qkpool.tile([64, HPC, L], bf16, name=f"vt{b}") for b in range(B)]

        vpool = ctx.enter_context(tc.tile_pool(name="vtm", bufs=1))
        v_sb = [vpool.tile([128, HPC, MC, 65], bf16, tag=f"v{b}", name=f"v_sb{b}")
                for b in range(B)]
        vdp = ctx.enter_context(tc.tile_pool(name="vdense", bufs=2))

        # o-proj tiles (at_sb half-loads are emitted mid-stream)
        wop = ctx.enter_context(tc.tile_pool(name="wo", bufs=1))
        wo_sb = wop.tile([128, KC, D], bf16)
        atp = ctx.enter_context(tc.tile_pool(name="at", bufs=1))
        at_sb = [atp.tile([128, KH, TPC], bf16, name=f"at_sb{h}")
                 for h in range(HPC)]

        # attention epilogue + pt pools (shared by early + main attention)
        epool = ctx.enter_context(tc.tile_pool(name="ep", bufs=2))
        ptpool = ctx.enter_context(tc.tile_pool(name="pt", bufs=4))

        att_outU = {}

        def attn_steps(h, b, nh, FQ, stp, oup, m0, m1):
            """Emit m-steps [m0, m1) of one (head, batch, qtile) attention
            block: scores, exp, A@V; epilogue + staging at the last step."""
            hs = slice(h * 64, (h + 1) * 64)
            NQ = min(512, FQ)
            q0 = nh * FQ
            if m0 == 0:
                att_outU[(h, b, nh)] = oup.tile([65, FQ], f32, tag="outU",
                                                name=f"oU{h}{b}{nh}")
            outU = att_outU[(h, b, nh)]
            for m in range(m0, m1):
                ks = slice(m * 128, (m + 1) * 128)
                st = stp.tile([128, FQ], f32, tag="st")
                for n in range(FQ // NQ):
                    qs = slice(q0 + n * NQ, q0 + (n + 1) * NQ)
                    nc.tensor.matmul(st[:, n * NQ:(n + 1) * NQ],
                                     qkt[b][hs, 1, ks], qkt[b][hs, 0, qs],
                                     start=True, stop=True)
                pt = ptpool.tile([128, FQ], bf16, tag="pt")
                nc.scalar.activation(pt[:], st[:], AF.Exp, scale=scale)
                for n in range(FQ // NQ):
                    ns = slice(n * NQ, (n + 1) * NQ)
                    nc.tensor.matmul(outU[:, ns],
                                     v_sb[b][:, h, m, :], pt[:, ns],
                                     start=(m == 0), stop=(m == MC - 1))
            if m1 < MC:
                return
            del att_outU[(h, b, nh)]
            # epilogue: normalize (DVE+Pool, off the PE/ACT path)
            dsb = epool.tile([1, FQ], f32, tag="dsb")
            nc.vector.reciprocal(dsb[:], outU[64:65, :])
            bc_sb = epool.tile([64, FQ], f32, tag="bc_sb")
            nc.gpsimd.partition_broadcast(bc_sb[:], dsb[:])
            a_st = epool.tile([64, FQ], bf16, tag="a_st")
            nc.vector.tensor_mul(a_st[:], outU[0:64, :], bc_sb[:])
            for nq in range(FQ // TPC):
                j = (b * L + nh * FQ) // TPC + nq
                nc.sync.dma_start(
                    a2a_in[h][j * 64:(j + 1) * 64, :],
                    a_st[:, nq * TPC:(nq + 1) * TPC])

        def attn_block(h, b, nh, FQ, stp, oup):
            attn_steps(h, b, nh, FQ, stp, oup, 0, MC)

        # ---------------- projections (+RoPE) ----------------
        estack = ExitStack()
        stpE = estack.enter_context(tc.tile_pool(name="stpE", bufs=3, space="PSUM"))
        oupE = estack.enter_context(tc.tile_pool(name="oupE", bufs=2, space="PSUM"))
        pstack = ExitStack()
        xtp = pstack.enter_context(tc.tile_pool(name="xt", bufs=2))
        up = pstack.enter_context(tc.tile_pool(name="u", bufs=2))
        t1p = pstack.enter_context(tc.tile_pool(name="t1", bufs=2))
        rup = pstack.enter_context(tc.tile_pool(name="ru", bufs=2))
        pps = pstack.enter_context(tc.tile_pool(name="pps", bufs=1, space="PSUM"))
        onc = pstack.enter_context(tc.tile_pool(name="onc", bufs=1))
        ones_col = onc.tile([128, HPC, MC, 1], bf16)
        nc.gpsimd.memset(ones_col[:], 1.0)

        xtiles = {}
        xdmas = {}

        def load_chunk(i):
            b, l0, ch = chunks[i]
            g0 = b * L + l0
            xt_c = xtp.tile([128, KC, 512], bf16, tag="xt_c", name=f"xt_c{i}")
            xsrc = xt_d[:, g0:g0 + ch].rearrange("(kk p) c -> p kk c", kk=KC)
            xdmas[i] = nc.sync.dma_start(xt_c[:, :, 0:ch], xsrc)
            xtiles[i] = xt_c

        pend_tp = []

        def flush_tp():
            while pend_tp:
                pend_tp.pop(0)()

        def proj_chunk(i):
            b, l0, ch = chunks[i]
            sl = slice(l0, l0 + ch)
            flush_tp()
            xt_c = xtiles.pop(i)
            q_ps = pps.tile([128, 512], f32, tag="q_ps")
            k_ps = pps.tile([128, 512], f32, tag="k_ps")
            v_ps = pps.tile([128, 512], f32, tag="v_ps")
            for kk in range(KC):
                st_, sp_ = (kk == 0), (kk == KC - 1)
                nc.tensor.matmul(q_ps[:, 0:ch], wq_sb[:, kk, :], xt_c[:, kk, 0:ch],
                                 start=st_, stop=sp_)
                nc.tensor.matmul(k_ps[:, 0:ch], wk_sb[:, kk, :], xt_c[:, kk, 0:ch],
                                 start=st_, stop=sp_)
                nc.tensor.matmul(v_ps[:, 0:ch], wv_sb[:, kk, :], xt_c[:, kk, 0:ch],
                                 start=st_, stop=sp_)
            # RoPE evacuation: cos/sin muls on DVE, rotate_half via 4 plain
            # strip DMAs (SP/ACT issue) + one 2x-mode DVE add.
            tb = slice(l0, l0 + ch)
            u = up.tile([128, 2, 512], f32, tag="u")
            t1 = t1p.tile([128, 2, 512], f32, tag="t1")
            ru = rup.tile([128, 2, 512], f32, tag="ru")
            nc.vector.tensor_mul(u[:, 0, 0:ch], q_ps[:, 0:ch], sinp[:, tb])
            nc.vector.tensor_mul(u[:, 1, 0:ch], k_ps[:, 0:ch], sinp[:, tb])
            nc.vector.tensor_mul(t1[:, 0, 0:ch], q_ps[:, 0:ch], cost[:, tb])
            nc.vector.tensor_mul(t1[:, 1, 0:ch], k_ps[:, 0:ch], cost[:, tb])
            for hh in range(HPC):
                a, b_ = hh * 64, hh * 64 + 32
                c_ = hh * 64 + 64
                eng = nc.sync if hh == 0 else nc.gpsimd
                eng.dma_start(ru[a:b_, :, 0:ch], u[b_:c_, :, 0:ch])
                eng.dma_start(ru[b_:c_, :, 0:ch], u[a:b_, :, 0:ch])
            with nc.allow_low_precision(reason="fp32r rounding on RoPE add"):
                # split per head-pair: the h0 half only waits the SP strip
                # pair, not the (laggier) Pool pair
                nc.vector.tensor_add(qkt[b][0:64, :, sl], t1[0:64, :, 0:ch],
                                     ru[0:64, :, 0:ch])
                nc.vector.tensor_add(qkt[b][64:128, :, sl], t1[64:128, :, 0:ch],
                                     ru[64:128, :, 0:ch])
            for hh in range(HPC):
                nc.vector.tensor_copy(vt[b][:, hh, sl],
                                      v_ps[hh * 64:(hh + 1) * 64, 0:ch])

            # v -> token-major [keys, 64|1] once this batch is projected:
            # one xbar transpose-DMA per head (token t lands at partition
            # t%128, chunk t//128 -- exactly the m-chunk layout).
            if l0 == 0:
                nc.vector.tensor_copy(v_sb[b][:, :, :, 64:65],
                                      ones_col[:, :, :, :])
            if b == 0 or l0 + ch == L:
                # HW xbar transpose requires a dense output; copy into the
                # 65-stride layout with a 4x-mode DVE op after.  Batch 0
                # transposes per chunk so attention can trail projection.
                t0, t1_ = (l0, l0 + ch) if b == 0 else (0, L)
                m0_, m1_ = t0 // 128, t1_ // 128
                for hh in range(HPC):
                    vden = vdp.tile([128, MC, 64], bf16, tag="vden",
                                    name=f"vden{b}_{hh}_{l0}")
                    nc.sync.dma_start_transpose(vden[:, m0_:m1_, :],
                                                vt[b][:, hh, t0:t1_])
                    nc.vector.tensor_copy(v_sb[b][:, hh, m0_:m1_, 0:64],
                                          vden[:, m0_:m1_, :])

        NC0 = len(sched0)             # chunks in batch 0
        load_chunk(0)
        if len(chunks) > 1:
            load_chunk(1)
        # rope tables (bf16: half a MB total)
        nc.sync.dma_start(cost[:], cost_d.ap()[:, :])
        nc.scalar.dma_start(sinp[:], sinp_d.ap()[:, :])

        # batch-0 projection with (h0, b0) attention m-steps interleaved as
        # soon as their query tile and key chunks are projected (the PE
        # sequencer is in-order, so emission order IS overlap).
        adone = {}
        for i in range(NC0):
            if i + 2 < len(chunks):
                load_chunk(i + 2)
            proj_chunk(i)
            # attention trails projection by ONE chunk: chunk i-1's transposes
            # were flushed at the head of proj_chunk(i), so steps up to its
            # coverage are emittable without blocking the x-chunk feed.
            if i == 0:
                continue
            end_tok = chunks[i - 1][1] + chunks[i - 1][2]
            for nh in range(L // FQE):
                if (nh + 1) * FQE > end_tok:
                    break
                if nh >= 2 and adone.get(nh - 2, 0) < MC:
                    break   # only 2 outU slots: qtile nh-2 must retire first
                m_hi = end_tok // 128
                m_lo = adone.get(nh, 0)
                if m_hi > m_lo:
                    attn_steps(0, 0, nh, FQE, stpE, oupE, m_lo, m_hi)
                    adone[nh] = m_hi

        flush_tp()
        for nh in range(L // FQE):
            if nh >= 2 and adone.get(nh - 2, 0) < MC:
                break
            m_lo = adone.get(nh, 0)
            if m_lo < MC:
                attn_steps(0, 0, nh, FQE, stpE, oupE, m_lo, MC)
                adone[nh] = MC
        # remaining early attention: (h1, b0), concurrent with the batch-1
        # projection below (which uses the other PSUM banks).
        for nh in range(L // FQE):
            attn_block(1, 0, nh, FQE, stpE, oupE)

        for i in range(NC0, len(chunks)):
            if i + 2 < len(chunks):
                load_chunk(i + 2)
            proj_chunk(i)

        flush_tp()
        pstack.close()
        estack.close()

        # ---------------- main attention (head-outer) ----------------
        s5 = ExitStack()
        stp = s5.enter_context(tc.tile_pool(name="stp", bufs=2, space="PSUM"))
        oup = s5.enter_context(tc.tile_pool(name="oup", bufs=2, space="PSUM"))

        blocks = [(hh, bb) for hh in range(HPC) for bb in range(1, B)]
        done_h = {hh: L // FQE for hh in range(HPC)}   # batch 0 done early
        total_h = {hh: L // FQE + (B - 1) * (L // FQM) for hh in range(HPC)}
        for h, b in blocks:
            for nh in range(L // FQM):
                attn_block(h, b, nh, FQM, stp, oup)
            done_h[h] = done_h.get(h, 0) + L // FQM
            if h == 0 and b == B - 1:
                # o-proj weights: needed last; pin behind the final x chunk
                # so the scheduler can't hoist the 2MB load into the
                # latency-critical projection feed.
                from concourse.tile_rust import add_dep_helper
                wod = nc.sync.dma_start(
                    wo_sb[:], wot_d.rearrange("(kk p) c -> p kk c", kk=KC))
                add_dep_helper(wod.ins, xdmas[len(chunks) - 1].ins, sync=False,
                               reason="wo load after projection feed")
            if done_h[h] == total_h[h]:
                # all batches of this head staged -> reshard while the rest
                # of attention runs
                nc.gpsimd.collective_compute(
                    "AllToAll", ALU.bypass,
                    ins=[a2a_in[h].opt()], outs=[a2a_out[h].opt()],
                    replica_groups=rg,
                )
                nc.sync.dma_start(
                    at_sb[h][:],
                    a2a_out[h][:, :].rearrange("(j p) t -> p j t", j=KH))

        s5.close()

        if debug:
            with tc.tile_pool(name="dbgp", bufs=1) as dbgp:
                dcp = dbgp.tile([128, 2, L], f32)
                nc.vector.tensor_copy(dcp[:], qkt[0][:, :, :].bitcast(f32))
                nc.sync.dma_start(dbg_qkt[:, :, :], dcp[:])
                dv = dbgp.tile([128, HPC, MC, 65], f32)
                nc.vector.tensor_copy(dv[:], v_sb[0][:, :, :, :])
                nc.sync.dma_start(dbg_vsb[:, :, :, :], dv[:])
                da = dbgp.tile([128, KC, TPC], f32)
                nc.vector.tensor_copy(da[:, 0:KH, :], at_sb[0][:, :, :])
                nc.vector.tensor_copy(da[:, KH:KC, :], at_sb[1][:, :, :])
                nc.sync.dma_start(dbg_at[:, :, :], da[:])
                for hh in range(HPC):
                    dao = dbgp.tile([64, N_CORES, TPC], f32, tag="dao", name=f"dao{hh}")
                    nc.gpsimd.dma_start(
                        dao[:],
                        a2a_out[hh][:, :].rearrange("(c p) t -> p c t", c=N_CORES))
                    nc.sync.dma_start(
                        [dbg_ao0, dbg_ao1][hh].rearrange("(c p) t -> p c t", c=N_CORES),
                        dao[:])

        # ---------------- o-projection (bf16, wo rows permuted) ----------
        # Two mt-waves of 4 PSUM banks; the other 4 banks host warm-keeper
        # matmuls so PE doesn't drop to the cold p-state while the head-1
        # AllToAll is still in flight (same reason HW HAM wants no PE idle).
        with tc.tile_pool(name="ysb", bufs=2) as ysp, \
             tc.tile_pool(name="yps", bufs=1, space="PSUM") as yps, \
             tc.tile_pool(name="warm", bufs=1, space="PSUM") as wrm:
            nyo = D // 512
            NMT = TPC // MT
            W1 = (NMT + 1) // 2
            y_ps = {}
            for mt in range(NMT):
                for no in range(nyo):
                    # waves share PSUM slots: wave-1 tiles reuse wave-0 banks
                    y_ps[(mt, no)] = yps.tile([MT, 512], f32,
                                              tag=f"y{mt % W1}_{no}",
                                              name=f"y_ps{mt}_{no}")
            warm_ps = wrm.tile([MT, 512], f32)

            def oproj_half(h, mts):
                for mt in mts:
                    ms = slice(mt * MT, (mt + 1) * MT)
                    for j in range(KH):
                        kk = h * KH + j
                        for no in range(nyo):
                            nc.tensor.matmul(
                                y_ps[(mt, no)][:], at_sb[h][:, j, ms],
                                wo_sb[:, kk, no * 512:(no + 1) * 512],
                                start=(kk == 0), stop=(kk == KC - 1))

            def evac(mts):
                for mt in mts:
                    ms = slice(mt * MT, (mt + 1) * MT)
                    y_sb = ysp.tile([MT, D], f32, tag="y_sb")
                    for no in range(nyo):
                        nc.vector.tensor_copy(y_sb[:, no * 512:(no + 1) * 512],
                                              y_ps[(mt, no)][:])
                    nc.sync.dma_start(y_d[ms, :], y_sb[:])

            wave0 = list(range(W1))
            wave1 = list(range(W1, NMT))
            oproj_half(0, wave0)
            for _ in range(100):   # PE keep-warm while the head-1 A2A flies
                nc.tensor.matmul(warm_ps[:], at_sb[0][:, 0, 0:MT],
                                 wo_sb[:, 0, 0:512], start=True, stop=True)
            oproj_half(1, wave0)
            evac(wave0)
            oproj_half(0, wave1)
            oproj_half(1, wave1)
            evac(wave1)

    return nc


def make_in_maps(x, wq, wk, wv, wo, L=L_FULL):
    import ml_dtypes
    bf = ml_dtypes.bfloat16
    T = B * L
    x2 = np.asarray(x, dtype=np.float32).reshape(T, D)
    xt_full = np.ascontiguousarray(x2.T.astype(bf))    # [D, T] bf16
    wq = np.asarray(wq, dtype=np.float32)
    wk = np.asarray(wk, dtype=np.float32)
    wv = np.asarray(wv, dtype=np.float32)
    wo = np.asarray(wo, dtype=np.float32)
    # Permute wo^T rows into a2a-output order: head-0 halves of all cores
    # first (64 rows per core), then head-1 halves.
    perm = np.empty(D, dtype=np.int64)
    for h in range(HPC):
        for rk in range(N_CORES):
            i0 = h * (N_CORES * 64) + rk * 64
            perm[i0:i0 + 64] = np.arange(rk * 128 + h * 64,
                                         rk * 128 + h * 64 + 64)
    wot_bf = np.ascontiguousarray(wo.T[perm]).astype(bf)
    in_maps = []
    for rk in range(N_CORES):
        rows = slice(rk * 128, (rk + 1) * 128)
        in_maps.append({
            "xt": xt_full,
            "wqt": np.ascontiguousarray(wq[rows].T).astype(bf),
            "wkt": np.ascontiguousarray(wk[rows].T).astype(bf),
            "wvt": np.ascontiguousarray(wv[rows].T).astype(bf),
            "wot": wot_bf,
        })
    return in_maps


_BUILT = {}


def _get_nc(L=L_FULL):
    if L not in _BUILT:
        import concourse.tile as tile
        from concourse import bacc
        nc = bacc.Bacc(num_devices=N_CORES)
        with tile.TileContext(nc) as tc:
            build_mha(tc, L=L)
        nc.compile()
        _BUILT[L] = nc
    return _BUILT[L]


def kernel(x, wq, wk, wv, wo):
    from concourse.bass_utils import run_bass_kernel_spmd
    nc = _get_nc()
    in_maps = make_in_maps(x, wq, wk, wv, wo)
    res = run_bass_kernel_spmd(nc, in_maps, core_ids=list(range(N_CORES)))
    y = np.concatenate([res.results[rk]["y"] for rk in range(N_CORES)], axis=0)
    return y.reshape(B, L_FULL, D)


# revision 36
# speedup vs baseline: 124.6111x; 62.3805x over previous
"""Trainium2 Bass kernel for 16-head MHA with RoPE (dense_transformer).

Sharding: tensor-parallel over heads (2 heads/core on 8 cores) for
QKV projection + attention, then an AllToAll resharding to
token-parallel (512 tokens/core) for the output projection.

v6 layout strategy (per core, rank r):
  - The full dim-major activation xT [1024, 4096] is fed to every core
    directly as a kernel input (host transposes once, bf16). This
    removes the AllGather + on-device transpose stage of v1 (~310us).
  - q/k are computed into one interleaved tile qkt [128, 2, L] f32 via
    bf16  wT.T @ xT  matmuls (fp32 PSUM accumulate), per-batch tiles.
    The first two projection chunks are half-sized so PE starts early.
  - RoPE: DVE multiplies by cos and a sign-folded sin table; the
    rotate_half partition swap is 4 plain strip DMAs (issued on the
    idle SP/ACT sequencers, q+k merged) into a scratch tile, then one
    2x-mode DVE add. No slow Pool-issued accumulate-DMAs.
  - v is evacuated bf16 and re-transposed to token-major [keys, 64|1]
    tiles by one xbar transpose-DMA per (batch, head); the appended
    ones column makes  out.T = [v | 1].T @ exp(S.T)  yield the softmax
    denominator as row 64 for free.
  - Attention is ScalarE(exp)-bound: batch-0/head-0 attention runs in
    small FQ=512 PSUM pools (4 banks) CONCURRENTLY with batch-1
    projection (4 banks); the remaining blocks use FQ=1024 (8 banks).
  - Softmax normalization: DVE reciprocal of the denominator row,
    GpSimd partition_broadcast (idle engine), DVE multiply -> bf16.
  - The head-parallel -> token-parallel reshard is TWO AllToAlls in
    bf16: the head-0 A2A overlaps head-1's attention; the o-projection
    is split so its head-0 half runs under the head-1 A2A (wo rows are
    host-permuted into a2a-output order to keep K-chunks contiguous).
"""

import numpy as np

# Problem shape (hardcoded per contract - kernel.py must be self-contained)
B, L_FULL, D = 2, 2048, 1024
H, HD = 16, 64
N_CORES = 8
HPC = H // N_CORES            # heads per core = 2
KC = D // 128                 # contraction chunks = 8


def _rope_tables(L):
    inv_freq = 1.0 / (10000.0 ** (np.arange(0, HD, 2, dtype=np.float64) / HD))
    t = np.arange(L, dtype=np.float64)
    freqs = np.outer(t, inv_freq)                      # [L, 32]
    emb = np.concatenate([freqs, freqs], -1)           # [L, 64]
    cos_t = np.cos(emb).T.astype(np.float32)           # [64, L]
    sin_t = np.sin(emb).T.astype(np.float32)
    cost = np.concatenate([cos_t, cos_t], 0)           # [128, L] (2 heads)
    sp = np.concatenate([sin_t[:32], -sin_t[32:]], 0)  # sign-folded
    sinp = np.concatenate([sp, sp], 0)                 # [128, L]
    return np.ascontiguousarray(cost), np.ascontiguousarray(sinp)


def build_mha(tc, L=L_FULL, debug=False):
    """Emit the MHA program into TileContext `tc`.

    Declares its own DRAM I/O tensors:
      in : xt [D, B*L] bf16 (full, replicated), wqt/wkt/wvt [D, 128] bf16,
           wot [D, D] bf16 (rows permuted to a2a-output order)
      out: y [B*L/8, D] f32
    """
    import concourse.bass as bass
    import concourse.mybir as mybir
    from contextlib import ExitStack

    nc = tc.nc
    f32 = mybir.dt.float32
    f32r = mybir.dt.float32r
    bf16 = mybir.dt.bfloat16
    AF = mybir.ActivationFunctionType
    ALU = mybir.AluOpType

    T = B * L                     # tokens
    TPC = T // N_CORES            # tokens per core (a2a shard width)
    MC = L // 128                 # key chunks per batch
    FQE = min(512, L)             # early-attention query tile (4 PSUM banks)
    FQM = min(1024, L)            # main attention query tile (8 PSUM banks)
    KH = N_CORES * 64 // 128      # o-proj K-chunks per head half = 4
    MT = min(128, TPC)            # o-proj token tile
    scale = float(HD) ** -0.5
    rg = [list(range(N_CORES))]

    # projection chunk schedule (within-batch offsets): the first chunks of
    # batch 0 are half-sized so the first matmul fires as early as possible.
    if L >= 512:
        sched0 = [(0, 256), (256, 256)] + [(o, 512) for o in range(512, L, 512)]
        schedN = [(o, 512) for o in range(0, L, 512)]
    else:
        sched0 = [(0, L)]
        schedN = [(0, L)]
    chunks = [(b, o, ch) for b in range(B)
              for (o, ch) in (sched0 if b == 0 else schedN)]

    def r(ap):
        return ap.bitcast(f32r)

    # ---- I/O ----
    xt_d = nc.dram_tensor("xt", [D, T], bf16, kind="ExternalInput").ap()
    wqt_d = nc.dram_tensor("wqt", [D, 128], bf16, kind="ExternalInput").ap()
    wkt_d = nc.dram_tensor("wkt", [D, 128], bf16, kind="ExternalInput").ap()
    wvt_d = nc.dram_tensor("wvt", [D, 128], bf16, kind="ExternalInput").ap()
    wot_d = nc.dram_tensor("wot", [D, D], bf16, kind="ExternalInput").ap()
    y_d = nc.dram_tensor("y", [TPC, D], f32, kind="ExternalOutput").ap()
    if debug:
        dbg_qkt = nc.dram_tensor("dbg_qkt", [128, 2, L], f32, kind="ExternalOutput").ap()
        dbg_vsb = nc.dram_tensor("dbg_vsb", [128, HPC, MC, 65], f32, kind="ExternalOutput").ap()
        dbg_ao0 = nc.dram_tensor("dbg_ao0", [N_CORES * 64, TPC], f32, kind="ExternalOutput").ap()
        dbg_ao1 = nc.dram_tensor("dbg_ao1", [N_CORES * 64, TPC], f32, kind="ExternalOutput").ap()
        dbg_at = nc.dram_tensor("dbg_at", [128, KC, TPC], f32, kind="ExternalOutput").ap()

    # ---- inline constants ----
    import ml_dtypes
    cost_np, sinp_np = _rope_tables(L)
    cost_d = nc.inline_tensor(cost_np.astype(ml_dtypes.bfloat16), name="cost")
    sinp_d = nc.inline_tensor(sinp_np.astype(ml_dtypes.bfloat16), name="sinp")

    ctx = ExitStack()
    with ctx:
        # ---------------- persistent pools ----------------
        wpool = ctx.enter_context(tc.tile_pool(name="wqkv", bufs=1))
        wq_sb = wpool.tile([128, KC, 128], bf16)
        wk_sb = wpool.tile([128, KC, 128], bf16)
        wv_sb = wpool.tile([128, KC, 128], bf16)
        nc.sync.dma_start(wq_sb[:], wqt_d.rearrange("(kk p) c -> p kk c", kk=KC))
        nc.scalar.dma_start(wk_sb[:], wkt_d.rearrange("(kk p) c -> p kk c", kk=KC))
        nc.gpsimd.dma_start(wv_sb[:], wvt_d.rearrange("(kk p) c -> p kk c", kk=KC))

        cpool = ctx.enter_context(tc.tile_pool(name="consts", bufs=1))
        cost = cpool.tile([128, L], bf16)
        sinp = cpool.tile([128, L], bf16)

        dram = ctx.enter_context(tc.tile_pool(name="dram", bufs=1, space="DRAM"))
        a2a_in = [dram.tile([N_CORES * 64, TPC], bf16, name=f"a2ai{h}")
                  for h in range(HPC)]
        a2a_out = [dram.tile([N_CORES * 64, TPC], bf16, name=f"a2ao{h}")
                   for h in range(HPC)]

        qkpool = ctx.enter_context(tc.tile_pool(name="qk", bufs=1))
        # per-batch tiles so batch-0 attention can overlap batch-1 work;
        # q and k interleaved so one strip DMA serves both
        qkt = [qkpool.tile([128, 2, L], f32r, name=f"qkt{b}") for b in range(B)]
        vt = [qkpool.tile([64, HPC, L], bf16, name=f"vt{b}") for b in range(B)]

        vpool = ctx.enter_context(tc.tile_pool(name="vtm", bufs=1))
        v_sb = [vpool.tile([128, HPC, MC, 65], bf16, tag=f"v{b}", name=f"v_sb{b}")
                for b in range(B)]
        vdp = ctx.enter_context(tc.tile_pool(name="vdense", bufs=2))

        # o-proj tiles (at_sb half-loads are emitted mid-stream)
        wop = ctx.enter_context(tc.tile_pool(name="wo", bufs=1))
        wo_sb = wop.tile([128, KC, D], bf16)
        atp = ctx.enter_context(tc.tile_pool(name="at", bufs=1))
        at_sb = [atp.tile([128, KH, TPC], bf16, name=f"at_sb{h}")
                 for h in range(HPC)]

        # attention epilogue + pt pools (shared by early + main attention)
        epool = ctx.enter_context(tc.tile_pool(name="ep", bufs=2))
        ptpool = ctx.enter_context(tc.tile_pool(name="pt", bufs=4))

        att_outU = {}

        def attn_steps(h, b, nh, FQ, stp, oup, m0, m1):
            """Emit m-steps [m0, m1) of one (head, batch, qtile) attention
            block: scores, exp, A@V; epilogue + staging at the last step."""
            hs = slice(h * 64, (h + 1) * 64)
            NQ = min(512, FQ)
            q0 = nh * FQ
            if m0 == 0:
                att_outU[(h, b, nh)] = oup.tile([65, FQ], f32, tag="outU",
                                                name=f"oU{h}{b}{nh}")
            outU = att_outU[(h, b, nh)]
            for m in range(m0, m1):
                ks = slice(m * 128, (m + 1) * 128)
                st = stp.tile([128, FQ], f32, tag="st")
                for n in range(FQ // NQ):
                    qs = slice(q0 + n * NQ, q0 + (n + 1) * NQ)
                    nc.tensor.matmul(st[:, n * NQ:(n + 1) * NQ],
                                     qkt[b][hs, 1, ks], qkt[b][hs, 0, qs],
                                     start=True, stop=True)
                pt = ptpool.tile([128, FQ], bf16, tag="pt")
                nc.scalar.activation(pt[:], st[:], AF.Exp, scale=scale)
                for n in range(FQ // NQ):
                    ns = slice(n * NQ, (n + 1) * NQ)
                    nc.tensor.matmul(outU[:, ns],
                                     v_sb[b][:, h, m, :], pt[:, ns],
                                     start=(m == 0), stop=(m == MC - 1))
            if m1 < MC:
                return
            del att_outU[(h, b, nh)]
            # epilogue: normalize (DVE+Pool, off the PE/ACT path)
            dsb = epool.tile([1, FQ], f32, tag="dsb")
            nc.vector.reciprocal(dsb[:], outU[64:65, :])
            bc_sb = epool.tile([64, FQ], f32, tag="bc_sb")
            nc.gpsimd.partition_broadcast(bc_sb[:], dsb[:])
            a_st = epool.tile([64, FQ], bf16, tag="a_st")
            nc.vector.tensor_mul(a_st[:], outU[0:64, :], bc_sb[:])
            for nq in range(FQ // TPC):
                j = (b * L + nh * FQ) // TPC + nq
                nc.sync.dma_start(
                    a2a_in[h][j * 64:(j + 1) * 64, :],
                    a_st[:, nq * TPC:(nq + 1) * TPC])

        def attn_block(h, b, nh, FQ, stp, oup):
            attn_steps(h, b, nh, FQ, stp, oup, 0, MC)

        # ---------------- projections (+RoPE) ----------------
        estack = ExitStack()
        stpE = estack.enter_context(tc.tile_pool(name="stpE", bufs=3, space="PSUM"))
        oupE = estack.enter_context(tc.tile_pool(name="oupE", bufs=2, space="PSUM"))
        pstack = ExitStack()
        xtp = pstack.enter_context(tc.tile_pool(name="xt", bufs=2))
        up = pstack.enter_context(tc.tile_pool(name="u", bufs=2))
        t1p = pstack.enter_context(tc.tile_pool(name="t1", bufs=2))
        rup = pstack.enter_context(tc.tile_pool(name="ru", bufs=2))
        pps = pstack.enter_context(tc.tile_pool(name="pps", bufs=1, space="PSUM"))
        onc = pstack.enter_context(tc.tile_pool(name="onc", bufs=1))
        ones_col = onc.tile([128, HPC, MC, 1], bf16)
        nc.gpsimd.memset(ones_col[:], 1.0)

        xtiles = {}
        xdmas = {}

        def load_chunk(i):
            b, l0, ch = chunks[i]
            g0 = b * L + l0
            xt_c = xtp.tile([128, KC, 512], bf16, tag="xt_c", name=f"xt_c{i}")
            xsrc = xt_d[:, g0:g0 + ch].rearrange("(kk p) c -> p kk c", kk=KC)
            xdmas[i] = nc.sync.dma_start(xt_c[:, :, 0:ch], xsrc)
            xtiles[i] = xt_c

        pend_tp = []
        pend_h1 = []

        def flush_tp():
            while pend_tp:
                pend_tp.pop(0)()
            while pend_h1:
                pend_h1.pop(0)()

        def proj_chunk(i):
            b, l0, ch = chunks[i]
            sl = slice(l0, l0 + ch)
            while pend_h1:
                pend_h1.pop(0)()
            flush_tp()
            xt_c = xtiles.pop(i)
            q_ps = pps.tile([128, 512], f32, tag="q_ps")
            k_ps = pps.tile([128, 512], f32, tag="k_ps")
            v_ps = pps.tile([128, 512], f32, tag="v_ps")
            for kk in range(KC):
                st_, sp_ = (kk == 0), (kk == KC - 1)
                nc.tensor.matmul(q_ps[:, 0:ch], wq_sb[:, kk, :], xt_c[:, kk, 0:ch],
                                 start=st_, stop=sp_)
                nc.tensor.matmul(k_ps[:, 0:ch], wk_sb[:, kk, :], xt_c[:, kk, 0:ch],
                                 start=st_, stop=sp_)
                nc.tensor.matmul(v_ps[:, 0:ch], wv_sb[:, kk, :], xt_c[:, kk, 0:ch],
                                 start=st_, stop=sp_)
            # RoPE evacuation: cos/sin muls on DVE, rotate_half via 4 plain
            # strip DMAs (SP/ACT issue) + one 2x-mode DVE add.
            tb = slice(l0, l0 + ch)
            u = up.tile([128, 2, 512], f32, tag="u")
            t1 = t1p.tile([128, 2, 512], f32, tag="t1")
            ru = rup.tile([128, 2, 512], f32, tag="ru")
            nc.vector.tensor_mul(u[:, 0, 0:ch], q_ps[:, 0:ch], sinp[:, tb])
            nc.vector.tensor_mul(u[:, 1, 0:ch], k_ps[:, 0:ch], sinp[:, tb])
            nc.vector.tensor_mul(t1[:, 0, 0:ch], q_ps[:, 0:ch], cost[:, tb])
            nc.vector.tensor_mul(t1[:, 1, 0:ch], k_ps[:, 0:ch], cost[:, tb])
            for hh in range(HPC):
                a, b_ = hh * 64, hh * 64 + 32
                c_ = hh * 64 + 64
                eng = nc.sync if hh == 0 else nc.gpsimd
                eng.dma_start(ru[a:b_, :, 0:ch], u[b_:c_, :, 0:ch])
                eng.dma_start(ru[b_:c_, :, 0:ch], u[a:b_, :, 0:ch])
            with nc.allow_low_precision(reason="fp32r rounding on RoPE add"):
                # split per head-pair: the h0 half only waits the SP strip
                # pair; the h1 half (gated on the laggier Pool strips) is
                # deferred one chunk so it doesn't block DVE's in-order queue
                nc.vector.tensor_add(qkt[b][0:64, :, sl], t1[0:64, :, 0:ch],
                                     ru[0:64, :, 0:ch])

                def emit_h1add(b=b, sl=sl, t1=t1, ru=ru, ch=ch):
                    with nc.allow_low_precision(reason="fp32r RoPE add h1"):
                        nc.vector.tensor_add(qkt[b][64:128, :, sl],
                                             t1[64:128, :, 0:ch],
                                             ru[64:128, :, 0:ch])
                pend_h1.append(emit_h1add)
            for hh in range(HPC):
                nc.vector.tensor_copy(vt[b][:, hh, sl],
                                      v_ps[hh * 64:(hh + 1) * 64, 0:ch])

            # v -> token-major [keys, 64|1] once this batch is projected:
            # one xbar transpose-DMA per head (token t lands at partition
            # t%128, chunk t//128 -- exactly the m-chunk layout).
            if l0 == 0:
                nc.vector.tensor_copy(v_sb[b][:, :, :, 64:65],
                                      ones_col[:, :, :, :])
            if b == 0 or l0 + ch == L:
                # HW xbar transpose requires a dense output; copy into the
                # 65-stride layout with a 4x-mode DVE op after.  Batch 0
                # transposes per chunk so attention can trail projection.
                t0, t1_ = (l0, l0 + ch) if b == 0 else (0, L)
                m0_, m1_ = t0 // 128, t1_ // 128
                for hh in range(HPC):
                    vden = vdp.tile([128, MC, 64], bf16, tag="vden",
                                    name=f"vden{b}_{hh}_{l0}")
                    nc.sync.dma_start_transpose(vden[:, m0_:m1_, :],
                                                vt[b][:, hh, t0:t1_])
                    nc.vector.tensor_copy(v_sb[b][:, hh, m0_:m1_, 0:64],
                                          vden[:, m0_:m1_, :])

        NC0 = len(sched0)             # chunks in batch 0
        load_chunk(0)
        if len(chunks) > 1:
            load_chunk(1)
        # rope tables (bf16: half a MB total)
        nc.sync.dma_start(cost[:], cost_d.ap()[:, :])
        nc.scalar.dma_start(sinp[:], sinp_d.ap()[:, :])

        # batch-0 projection with (h0, b0) attention m-steps interleaved as
        # soon as their query tile and key chunks are projected (the PE
        # sequencer is in-order, so emission order IS overlap).
        adone = {}
        for i in range(NC0):
            if i + 2 < len(chunks):
                load_chunk(i + 2)
            proj_chunk(i)
            # attention trails projection by ONE chunk: chunk i-1's transposes
            # were flushed at the head of proj_chunk(i), so steps up to its
            # coverage are emittable without blocking the x-chunk feed.
            if i == 0:
                continue
            end_tok = chunks[i - 1][1] + chunks[i - 1][2]
            for nh in range(L // FQE):
                if (nh + 1) * FQE > end_tok:
                    break
                if nh >= 2 and adone.get(nh - 2, 0) < MC:
                    break   # only 2 outU slots: qtile nh-2 must retire first
                m_hi = end_tok // 128
                m_lo = adone.get(nh, 0)
                if m_hi > m_lo:
                    attn_steps(0, 0, nh, FQE, stpE, oupE, m_lo, m_hi)
                    adone[nh] = m_hi

        flush_tp()
        for nh in range(L // FQE):
            if nh >= 2 and adone.get(nh - 2, 0) < MC:
                break
            m_lo = adone.get(nh, 0)
            if m_lo < MC:
                attn_steps(0, 0, nh, FQE, stpE, oupE, m_lo, MC)
                adone[nh] = MC
        # remaining early attention: (h1, b0), concurrent with the batch-1
        # projection below (which uses the other PSUM banks).
        for nh in range(L // FQE):
            attn_block(1, 0, nh, FQE, stpE, oupE)

        for i in range(NC0, len(chunks)):
            if i + 2 < len(chunks):
                load_chunk(i + 2)
            proj_chunk(i)

        flush_tp()
        pstack.close()
        estack.close()

        # ---------------- main attention (head-outer) ----------------
        s5 = ExitStack()
        stp = s5.enter_context(tc.tile_pool(name="stp", bufs=2, space="PSUM"))
        oup = s5.enter_context(tc.tile_pool(name="oup", bufs=2, space="PSUM"))

        blocks = [(hh, bb) for hh in range(HPC) for bb in range(1, B)]
        done_h = {hh: L // FQE for hh in range(HPC)}   # batch 0 done early
        total_h = {hh: L // FQE + (B - 1) * (L // FQM) for hh in range(HPC)}
        for h, b in blocks:
            for nh in range(L // FQM):
                attn_block(h, b, nh, FQM, stp, oup)
            done_h[h] = done_h.get(h, 0) + L // FQM
            if h == 0 and b == B - 1:
                # o-proj weights: needed last; pin behind the final x chunk
                # so the scheduler can't hoist the 2MB load into the
                # latency-critical projection feed.
                from concourse.tile_rust import add_dep_helper
                wod = nc.sync.dma_start(
                    wo_sb[:], wot_d.rearrange("(kk p) c -> p kk c", kk=KC))
                add_dep_helper(wod.ins, xdmas[len(chunks) - 1].ins, sync=False,
                               reason="wo load after projection feed")
            if done_h[h] == total_h[h]:
                # all batches of this head staged -> reshard while the rest
                # of attention runs
                nc.gpsimd.collective_compute(
                    "AllToAll", ALU.bypass,
                    ins=[a2a_in[h].opt()], outs=[a2a_out[h].opt()],
                    replica_groups=rg,
                )
                nc.sync.dma_start(
                    at_sb[h][:],
                    a2a_out[h][:, :].rearrange("(j p) t -> p j t", j=KH))

        s5.close()

        if debug:
            with tc.tile_pool(name="dbgp", bufs=1) as dbgp:
                dcp = dbgp.tile([128, 2, L], f32)
                nc.vector.tensor_copy(dcp[:], qkt[0][:, :, :].bitcast(f32))
                nc.sync.dma_start(dbg_qkt[:, :, :], dcp[:])
                dv = dbgp.tile([128, HPC, MC, 65], f32)
                nc.vector.tensor_copy(dv[:], v_sb[0][:, :, :, :])
                nc.sync.dma_start(dbg_vsb[:, :, :, :], dv[:])
                da = dbgp.tile([128, KC, TPC], f32)
                nc.vector.tensor_copy(da[:, 0:KH, :], at_sb[0][:, :, :])
                nc.vector.tensor_copy(da[:, KH:KC, :], at_sb[1][:, :, :])
                nc.sync.dma_start(dbg_at[:, :, :], da[:])
                for hh in range(HPC):
                    dao = dbgp.tile([64, N_CORES, TPC], f32, tag="dao", name=f"dao{hh}")
                    nc.gpsimd.dma_start(
                        dao[:],
                        a2a_out[hh][:, :].rearrange("(c p) t -> p c t", c=N_CORES))
                    nc.sync.dma_start(
                        [dbg_ao0, dbg_ao1][hh].rearrange("(c p) t -> p c t", c=N_CORES),
                        dao[:])

        # ---------------- o-projection (bf16, wo rows permuted) ----------
        # Two mt-waves of 4 PSUM banks; the other 4 banks host warm-keeper
        # matmuls so PE doesn't drop to the cold p-state while the head-1
        # AllToAll is still in flight (same reason HW HAM wants no PE idle).
        with tc.tile_pool(name="ysb", bufs=2) as ysp, \
             tc.tile_pool(name="yps", bufs=1, space="PSUM") as yps, \
             tc.tile_pool(name="warm", bufs=1, space="PSUM") as wrm:
            nyo = D // 512
            NMT = TPC // MT
            W1 = (NMT + 1) // 2
            y_ps = {}
            for mt in range(NMT):
                for no in range(nyo):
                    # waves share PSUM slots: wave-1 tiles reuse wave-0 banks
                    y_ps[(mt, no)] = yps.tile([MT, 512], f32,
                                              tag=f"y{mt % W1}_{no}",
                                              name=f"y_ps{mt}_{no}")
            warm_ps = wrm.tile([MT, 512], f32)

            def oproj_half(h, mts):
                for mt in mts:
                    ms = slice(mt * MT, (mt + 1) * MT)
                    for j in range(KH):
                        kk = h * KH + j
                        for no in range(nyo):
                            nc.tensor.matmul(
                                y_ps[(mt, no)][:], at_sb[h][:, j, ms],
                                wo_sb[:, kk, no * 512:(no + 1) * 512],
                                start=(kk == 0), stop=(kk == KC - 1))

            def evac(mts):
                for mt in mts:
                    ms = slice(mt * MT, (mt + 1) * MT)
                    y_sb = ysp.tile([MT, D], f32, tag="y_sb")
                    for no in range(nyo):
                        nc.vector.tensor_copy(y_sb[:, no * 512:(no + 1) * 512],
                                              y_ps[(mt, no)][:])
                    nc.sync.dma_start(y_d[ms, :], y_sb[:])

            wave0 = list(range(W1))
            wave1 = list(range(W1, NMT))
            oproj_half(0, wave0)
            for _ in range(100):   # PE keep-warm while the head-1 A2A flies
                nc.tensor.matmul(warm_ps[:], at_sb[0][:, 0, 0:MT],
                                 wo_sb[:, 0, 0:512], start=True, stop=True)
            oproj_half(1, wave0)
            evac(wave0)
            oproj_half(0, wave1)
            oproj_half(1, wave1)
            evac(wave1)

    return nc


def make_in_maps(x, wq, wk, wv, wo, L=L_FULL):
    import ml_dtypes
    bf = ml_dtypes.bfloat16
    T = B * L
    x2 = np.asarray(x, dtype=np.float32).reshape(T, D)
    xt_full = np.ascontiguousarray(x2.T.astype(bf))    # [D, T] bf16
    wq = np.asarray(wq, dtype=np.float32)
    wk = np.asarray(wk, dtype=np.float32)
    wv = np.asarray(wv, dtype=np.float32)
    wo = np.asarray(wo, dtype=np.float32)
    # Permute wo^T rows into a2a-output order: head-0 halves of all cores
    # first (64 rows per core), then head-1 halves.
    perm = np.empty(D, dtype=np.int64)
    for h in range(HPC):
        for rk in range(N_CORES):
            i0 = h * (N_CORES * 64) + rk * 64
            perm[i0:i0 + 64] = np.arange(rk * 128 + h * 64,
                                         rk * 128 + h * 64 + 64)
    wot_bf = np.ascontiguousarray(wo.T[perm]).astype(bf)
    in_maps = []
    for rk in range(N_CORES):
        rows = slice(rk * 128, (rk + 1) * 128)
        in_maps.append({
            "xt": xt_full,
            "wqt": np.ascontiguousarray(wq[rows].T).astype(bf),
            "wkt": np.ascontiguousarray(wk[rows].T).astype(bf),
            "wvt": np.ascontiguousarray(wv[rows].T).astype(bf),
            "wot": wot_bf,
        })
    return in_maps


_BUILT = {}


def _get_nc(L=L_FULL):
    if L not in _BUILT:
        import concourse.tile as tile
        from concourse import bacc
        nc = bacc.Bacc(num_devices=N_CORES)
        with tile.TileContext(nc) as tc:
            build_mha(tc, L=L)
        nc.compile()
        _BUILT[L] = nc
    return _BUILT[L]


def kernel(x, wq, wk, wv, wo):
    from concourse.bass_utils import run_bass_kernel_spmd
    nc = _get_nc()
    in_maps = make_in_maps(x, wq, wk, wv, wo)
    res = run_bass_kernel_spmd(nc, in_maps, core_ids=list(range(N_CORES)))
    y = np.concatenate([res.results[rk]["y"] for rk in range(N_CORES)], axis=0)
    return y.reshape(B, L_FULL, D)
